# revision 37
# baseline (speedup 1.0000x reference)
"""Trainium2 Bass kernel: bidirectional GNN message passing (scatter-add) + concat.

Computation (per batch b):
    out[b, :, 0:256]   = M_b @ x[b]        where M_b[i, j] = (# edges i<-j) + (# edges j<-i)
    out[b, :, 256:512] = x[b]

M_b is a symmetric count matrix built on the host from the edge indices (pure
index preprocessing; all x-dependent arithmetic runs on the NeuronCores).
Sharding: data-parallel over the batch dim, 4 batches per core on 8 cores.
On-device the scatter-add is computed as dense 128x128-block matmuls on the
tensor engine (f16 x f16 -> fp32 PSUM accumulation over the 16 source-node
blocks; counts are exact in f16, x is rounded to f16 on the DVE).
"""

import numpy as np

B, N, D = 32, 2048, 256
NC = 8                  # cores
BPC = B // NC           # batches per core = 4
NB = N // 128           # node blocks per batch = 16
G = BPC * NB            # node blocks per core = 64
AMERGE = 1              # strips per A DMA
OMERGE = 2              # strips per out DMA

_compiled = None


def _build_bass():
    from contextlib import ExitStack
    import concourse.bass as bass
    import concourse.tile as tile
    from concourse import bacc, mybir

    nc = bacc.Bacc("TRN2", target_bir_lowering=False, debug=False, num_devices=NC)
    x_ap = nc.dram_tensor("x", [BPC * N, D], mybir.dt.float32, kind="ExternalInput").ap()
    # A layout [b, im, s, ii, J, d] u8: each im-group of AMERGE dst-strips is a
    # flat [128, AMERGE*NB*128] block -> 8KB-contiguous DMA descriptor runs.
    a_ap = nc.dram_tensor(
        "a", [BPC, NB // AMERGE, 128, AMERGE * NB * 128], mybir.dt.uint8, kind="ExternalInput"
    ).ap()
    out_ap = nc.dram_tensor("out", [BPC * N, 2 * D], mybir.dt.float32, kind="ExternalOutput").ap()

    with tile.TileContext(nc) as tc:
        with ExitStack() as ctx:
            xpool = ctx.enter_context(tc.tile_pool(name="x", bufs=1))
            xhpool = ctx.enter_context(tc.tile_pool(name="xh", bufs=1))
            apool = ctx.enter_context(tc.tile_pool(name="a8", bufs=5))
            afpool = ctx.enter_context(tc.tile_pool(name="af", bufs=6))
            pspool = ctx.enter_context(tc.tile_pool(name="ps", bufs=4, space="PSUM"))
            opool = ctx.enter_context(tc.tile_pool(name="o", bufs=3))

            # x resident in SBUF: [p, (g, d)] where node n = g*128 + p.
            # Loaded per batch, interleaved into the A-load stream (FIFO ring)
            # so batch 0's strips start immediately and batch b+1's x arrives
            # while batch b computes.
            x_sb = xpool.tile([128, G * D], mybir.dt.float32)
            x_h = xhpool.tile([128, G * D], mybir.dt.float16)
            xw = NB * D  # per-batch width

            def load_x(b, q):
                # quarter-batch granularity: earlier first matmul, smoother DMA
                qw = xw // 4
                lo = b * xw + q * qw
                n0 = b * N + q * (N // 4)
                nc.sync.dma_start(
                    x_sb[:, lo : lo + qw],
                    x_ap[n0 : n0 + N // 4].rearrange("(g p) d -> p g d", p=128),
                )
                nc.vector.tensor_copy(x_h[:, lo : lo + qw], x_sb[:, lo : lo + qw])

            first_a = {}
            for b in range(BPC):
                for im in range(NB // AMERGE):
                    # one DMA covering AMERGE dst-strips of A (u8)
                    a_t = apool.tile([128, AMERGE * NB * 128], mybir.dt.uint8)
                    nc.sync.dma_start(a_t[:], a_ap[b, im])
                    if b == 0 and im == 0:
                        for q in range(4):
                            load_x(0, q)  # behind the first A load on the ring
                    if b + 1 < BPC and im % 4 == 0:
                        load_x(b + 1, im // 4)  # prefetch next batch's x, one quarter per 4 im
                    if im == (7 if b == BPC - 1 else 15):
                        # x-half of the output for this batch: straight SBUF ->
                        # HBM via the SWDGE ring, emitted late in the batch so
                        # it runs in DMA slack (earlier for the last batch so it
                        # doesn't extend the tail).
                        nc.gpsimd.dma_start(
                            out_ap[b * N : (b + 1) * N, D:].rearrange("(g p) d -> p g d", p=128),
                            x_sb[:, b * xw : (b + 1) * xw],
                        )
                    for ii in range(AMERGE):
                        i = im * AMERGE + ii
                        g = b * NB + i
                        # cast strip u8 -> f16, 4 alternating segments so the PE's
                        # in-order j consumption pipelines against both engines
                        a_f = afpool.tile([128, NB * 128], mybir.dt.float16)
                        asrc = a_t[:, ii * NB * 128 : (ii + 1) * NB * 128]
                        segs = [(0, 5, nc.scalar), (5, 8, nc.vector), (8, 13, nc.scalar), (13, 16, nc.vector)]
                        for s0, s1, eng in segs:
                            c0, c1 = s0 * 128, s1 * 128
                            if eng is nc.scalar:
                                nc.scalar.copy(a_f[:, c0:c1], asrc[:, c0:c1])
                            else:
                                nc.vector.tensor_copy(a_f[:, c0:c1], asrc[:, c0:c1])
                        pt = pspool.tile([128, D], mybir.dt.float32)
                        for j in range(NB):
                            nc.tensor.matmul(
                                pt[:],
                                a_f[:, j * 128 : (j + 1) * 128],
                                x_h[:, (b * NB + j) * D : (b * NB + j + 1) * D],
                                start=(j == 0),
                                stop=(j == NB - 1),
                            )
                        if i % OMERGE == 0:
                            o_t = opool.tile([128, OMERGE * D], mybir.dt.float32)
                        oo = i % OMERGE
                        nc.vector.tensor_copy(o_t[:, oo * D : (oo + 1) * D], pt[:])
                        if i % OMERGE == OMERGE - 1:
                            g0 = b * NB + i - (OMERGE - 1)
                            nc.gpsimd.dma_start(
                                out_ap[g0 * 128 : (g0 + OMERGE) * 128, :D].rearrange(
                                    "(gg p) c -> p gg c", p=128
                                ),
                                o_t[:],
                            )

    nc.compile()
    return nc


def _host_build_adjacency(batch_idx, src_idx, dst_idx):
    """Per-batch symmetric count matrices, laid out as lhsT blocks.

    Returns u8 array [B, NB//AMERGE, 128, AMERGE, NB, 128]: a[b, im, s, ii, j, d]
    = M_b[j*128+s, (im*AMERGE+ii)*128+d] (M symmetric: [src, dst] block feeding
    dst-block im*AMERGE+ii from src-block j), im-group contiguous per s for DMA.
    """
    a = np.empty((B, NB // AMERGE, 128, AMERGE, NB, 128), dtype=np.uint8)
    order = np.argsort(batch_idx, kind="stable")
    bcounts = np.bincount(batch_idx.astype(np.int64), minlength=B)
    offs = np.zeros(B + 1, dtype=np.int64)
    np.cumsum(bcounts, out=offs[1:])
    src_s = src_idx[order].astype(np.int64)
    dst_s = dst_idx[order].astype(np.int64)
    for b in range(B):
        s = src_s[offs[b] : offs[b + 1]]
        d = dst_s[offs[b] : offs[b + 1]]
        ids = np.concatenate([d * N + s, s * N + d])
        m = np.bincount(ids, minlength=N * N)
        # m[row, col]: row = src (lhsT partition), col = dst (M symmetric)
        mr = m.reshape(NB, 128, NB, 128)  # [J, s, I, d]
        isd = mr.transpose(2, 1, 0, 3).astype(np.uint8)  # [I, s, J, d]
        a[b] = isd.reshape(NB // AMERGE, AMERGE, 128, NB, 128).transpose(0, 2, 1, 3, 4)
    return a


def kernel(x, batch_idx, src_idx, dst_idx):
    global _compiled
    from concourse import bass_utils

    assert x.shape == (B, N, D), x.shape
    a_all = _host_build_adjacency(batch_idx, src_idx, dst_idx)

    if _compiled is None:
        _compiled = _build_bass()
    nc = _compiled

    in_maps = []
    for c in range(NC):
        xs = np.ascontiguousarray(
            x[c * BPC : (c + 1) * BPC].reshape(BPC * N, D).astype(np.float32)
        )
        asrd = np.ascontiguousarray(a_all[c * BPC : (c + 1) * BPC])
        in_maps.append({"x": xs, "a": asrd})

    res = bass_utils.run_bass_kernel_spmd(nc, in_maps, core_ids=list(range(NC)))

    out = np.empty((B, N, 2 * D), dtype=np.float32)
    for c in range(NC):
        out[c * BPC : (c + 1) * BPC] = res.results[c]["out"].reshape(BPC, N, 2 * D)
    return out
